# revision 1
# baseline (speedup 1.0000x reference)
"""Trainium2 Bass kernel for nn_CausalSelfAttention_15178414424258.

GQA sliding-window causal attention (HQ=16, HK=4, D=64, WINDOW=1024) with
value-embedding gating, rope + qk rms-norm, out-projection.

Sharding: tensor-parallel over the 4 kv-head groups x data-parallel over the
2 batches = 8 cores. Each core handles one batch b and one kv group g
(4 q heads, 1 k head, 1 v head), produces a partial out-projection
(its 256 channels of the attention output against the matching w_o columns);
the host sums the 4 partials per batch.

On-core dataflow (all matmuls fp32r; scores kept transposed [t_k, t_q] so
softmax denominators come free via a ones-column appended to V):
  A) qkv = x @ w_qkv_shard^T (+ gate logit col), gate/sigmoid, v += gate*ve,
     rope via [x1|x1]*[c|s] + [x2|x2]*[-s|c], rms-norm via Ln/Exp rsqrt,
     PE-transpose q/k to [d, t].
  B) per 512-query chunk and head-pair: S^T = k^T q in PSUM, exp on ACT
     (scale=1/8 folds the 1/sqrt(D)), mask/zero edge quarters on GPSIMD,
     PV accumulate with ones-augmented V giving aoT + denom row,
     reciprocal + partition-broadcast-DMA + normalize into aoT.
  C) out_partial = aoT^T @ w_oT_shard, DMA to DRAM.
"""
import sys

sys.path.insert(0, "/opt/trn_rl_repo")

from contextlib import ExitStack  # noqa: E402

import numpy as np  # noqa: E402

import concourse.bass as bass  # noqa: E402
import concourse.tile as tile  # noqa: E402
from concourse import bacc, mybir  # noqa: E402
from concourse.bass_utils import run_bass_kernel_spmd  # noqa: E402

F32 = mybir.dt.float32
F32R = mybir.dt.float32r
BF16 = mybir.dt.bfloat16
AF = mybir.ActivationFunctionType
ALU = mybir.AluOpType
AX = mybir.AxisListType

B, T, E = 2, 2048, 1024
HQ, HK, D = 16, 4, 64
WINDOW = 1024
GATE_CH = 12
RMS_EPS = 1e-8
G = HQ // HK          # 4 q heads per kv group
TB = T // 128         # 16 t-blocks
NC_ = 4               # 512-wide query chunks
KT = E // 128         # 8 k-tiles for the qkv matmul

_CACHE = {}


def _active_m(c):
    return range(max(0, 4 * c - 8), 4 * c + 4)


def _pin_act_tables(nc):
    """Keep Exp/Ln only in the combined set so insert_act_table_loads
    emits a single table load instead of thrashing between sets."""
    from concourse import hw_specs
    tabs = hw_specs.get_activation_tables(nc.m.arch)
    for name, s in tabs.items():
        if name != "natural_log_exp_and_others":
            s.discard(AF.Exp)
            s.discard(AF.Ln)


def build_program():
    nc = bacc.Bacc("TRN2", target_bir_lowering=False, debug=False, num_devices=8)
    _pin_act_tables(nc)

    xT = nc.declare_dram_parameter("xT", [E, T], F32R, isOutput=False)
    wqkvT = nc.declare_dram_parameter("wqkvT", [E, 386], F32R, isOutput=False)
    ve3 = nc.declare_dram_parameter("ve3", [T, D], F32, isOutput=False)
    ropeA = nc.declare_dram_parameter("ropeA", [T, D], F32, isOutput=False)
    ropeB = nc.declare_dram_parameter("ropeB", [T, D], F32, isOutput=False)
    woT = nc.declare_dram_parameter("woT", [G * D, E], F32R, isOutput=False)
    maskC = nc.declare_dram_parameter("maskC", [128, 128], BF16, isOutput=False)
    maskW = nc.declare_dram_parameter("maskW", [128, 128], BF16, isOutput=False)
    ident = nc.declare_dram_parameter("ident", [128, 128], F32R, isOutput=False)
    identb = nc.declare_dram_parameter("identb", [128, 128], BF16, isOutput=False)
    out = nc.declare_dram_parameter("out", [T, E], F32, isOutput=True)

    with tile.TileContext(nc) as tc, ExitStack() as ctx:
        P = lambda **kw: ctx.enter_context(tc.tile_pool(**kw))
        pers = P(name="pers", bufs=1)
        xp = P(name="xp", bufs=2)
        tmp = P(name="tmp", bufs=2)
        p2p = P(name="p2p", bufs=6)
        outs = P(name="outs", bufs=3)
        # PSUM budget (8 banks): tag "s" 2x[128,1024] (scores + qkv/outproj),
        # tags "a0"/"a1" 1x[128,1024] each (PV accumulators, hp-alternating,
        # reused for the phase-A transposes)
        ps = P(name="ps", bufs=1, space="PSUM")

        # ---- persistent SBUF ----
        wq_sb = [pers.tile([128, 386], F32R, tag=f"wq{k}", name=f"wq{k}") for k in range(KT)]
        wo_sb = [pers.tile([128, E], F32R, tag=f"wo{k}", name=f"wo{k}") for k in range(2)]
        ra_sb = pers.tile([128, TB, D], F32, tag="ra")
        rb_sb = pers.tile([128, TB, D], F32, tag="rb")
        ve_sb = pers.tile([128, TB, D], F32, tag="ve")
        mc_sb = pers.tile([128, 128], BF16, tag="mc")
        mw_sb = pers.tile([128, 128], BF16, tag="mw")
        v1a = pers.tile([128, TB, 128], F32R, tag="v1a")   # [v | 1 | 0...]
        v1b = pers.tile([128, TB, 128], F32R, tag="v1b")   # [0...| 1 | v]
        qt_sb = [pers.tile([128, T], F32R, tag=f"qt{p}", name=f"qt{p}") for p in range(2)]
        kt_sb = pers.tile([128, T], F32R, tag="kt")  # kT duplicated in both halves
        aot = [pers.tile([128, T], F32R, tag=f"aot{p}", name=f"aot{p}") for p in range(2)]

        wq_r = wqkvT.rearrange("(k p) f -> k p f", p=128)
        wo_r = woT.rearrange("(k p) f -> k p f", p=128)
        xT_r0 = xT.rearrange("(k p) t -> k p t", p=128)
        x_first = [xp.tile([128, 512], F32R, tag=f"x{k}", name=f"x{k}")
                   for k in range(KT)]
        for k in range(KT):
            nc.sync.dma_start(x_first[k][:], xT_r0[k, :, 0:512])
            nc.sync.dma_start(wq_sb[k][:], wq_r[k])
        nc.sync.dma_start(ra_sb[:], ropeA.rearrange("(tb p) d -> p tb d", p=128))
        nc.sync.dma_start(rb_sb[:], ropeB.rearrange("(tb p) d -> p tb d", p=128))
        nc.sync.dma_start(ve_sb[:], ve3.rearrange("(tb p) d -> p tb d", p=128))
        for k in range(2):
            nc.sync.dma_start(wo_sb[k][:], wo_r[k])
        nc.sync.dma_start(mc_sb[:], maskC[:])
        nc.sync.dma_start(mw_sb[:], maskW[:])

        # ones/zeros pattern of the augmented V copies
        nc.vector.memset(v1a[:].bitcast(F32), 0.0)
        nc.vector.memset(v1b[:].bitcast(F32), 0.0)
        for tb in range(TB):
            nc.vector.memset(v1a[:, tb, 64:65].bitcast(F32), 1.0)
            nc.vector.memset(v1b[:, tb, 63:64].bitcast(F32), 1.0)

        identity = pers.tile([128, 128], F32R, tag="ident")
        nc.sync.dma_start(identity[:], ident[:])
        identity_b = pers.tile([128, 128], BF16, tag="identb")
        nc.sync.dma_start(identity_b[:], identb[:])

        xT_r = xT.rearrange("(k p) t -> k p t", p=128)

        # ================= Phase A =================
        qn_kn = {}
        for tb in range(TB):
            c, r = divmod(tb, 4)
            if r == 0:
                if c == 0:
                    x_sb = x_first
                else:
                    x_sb = [xp.tile([128, 512], F32R, tag=f"x{k}",
                                    name=f"x{k}") for k in range(KT)]
                    for k in range(KT):
                        nc.sync.dma_start(x_sb[k][:],
                                          xT_r[k, :, c * 512:(c + 1) * 512])
            qkv_ps = ps.tile([128, 1024], F32, tag="s", name="qkv_ps",
                             bufs=2)[:, 0:512]
            for k in range(KT):
                nc.tensor.matmul(qkv_ps[:, 0:386],
                                 x_sb[k][:, r * 128:(r + 1) * 128],
                                 wq_sb[k][:], start=(k == 0), stop=(k == KT - 1))
            # PSUM -> SBUF once (ACT) so rope/v-gate can run on GPSIMD
            qkv = tmp.tile([128, 386], F32, tag="qkvs", bufs=3)
            nc.scalar.copy(qkv[:], qkv_ps[:, 0:386])

            # gate = sigmoid(logit) via 1/(1+exp(-x)); v = qkv_v + gate*ve3
            eg = tmp.tile([128, 1], F32, tag="eg")
            nc.scalar.activation(eg[:], qkv[:, 384:385], AF.Exp, scale=-1.0)
            gp = tmp.tile([128, 1], F32, tag="gp")
            nc.vector.tensor_scalar_add(gp[:], eg[:], 1.0)
            gi = tmp.tile([128, 1], F32, tag="gi")
            nc.vector.reciprocal_approx_fast(gi[:], gp[:])
            vt = tmp.tile([128, D], F32, tag="vt")
            nc.vector.tensor_scalar_mul(vt[:], ve_sb[:, tb], gi[:])
            nc.vector.tensor_add(v1a[:, tb, 0:64], qkv[:, 320:384], vt[:])
            nc.gpsimd.tensor_copy(v1b[:, tb, 64:128], v1a[:, tb, 0:64])

            # rope: out = [x1|x1]*[c|s] + [x2|x2]*[-s|c]
            def rope(dst, src_ap, nh, eng):
                x1 = src_ap[:, :, 0:32].unsqueeze(2).broadcast_to([128, nh, 2, 32])
                x2 = src_ap[:, :, 32:64].unsqueeze(2).broadcast_to([128, nh, 2, 32])
                rav = (ra_sb[:, tb].rearrange("p (two d) -> p two d", two=2)
                       .unsqueeze(1).broadcast_to([128, nh, 2, 32]))
                rbv = (rb_sb[:, tb].rearrange("p (two d) -> p two d", two=2)
                       .unsqueeze(1).broadcast_to([128, nh, 2, 32]))
                dv = dst[:].rearrange("p (h two d) -> p h two d", h=nh, two=2)
                t1 = tmp.tile([128, nh * 64], F32, tag=f"t1{nh}")
                t1v = t1[:].rearrange("p (h two d) -> p h two d", h=nh, two=2)
                eng.tensor_tensor(t1v, x1, rav, ALU.mult)
                eng.tensor_tensor(dv, x2, rbv, ALU.mult)
                eng.tensor_add(dst[:], dst[:], t1[:])

            qr = tmp.tile([128, G * D], F32, tag="qr")
            rope(qr, qkv[:, 0:256].rearrange("p (h d) -> p h d", h=G), G,
                 nc.gpsimd)
            kr = tmp.tile([128, D], F32, tag="kr")
            rope(kr, qkv[:, 256:320].rearrange("p (h d) -> p h d", h=1), 1,
                 nc.vector)

            # rms-norm scales: rsqrt(mean(x^2)+eps) = exp(-0.5*ln(m))
            sq = tmp.tile([128, D], F32, tag="sq")
            ss = tmp.tile([128, 8], F32, tag="ss")
            for h in range(G):
                nc.scalar.activation(sq[:], qr[:, h * 64:(h + 1) * 64],
                                     AF.Square, accum_out=ss[:, h:h + 1])
            nc.scalar.activation(sq[:], kr[:], AF.Square,
                                 accum_out=ss[:, 4:5])
            m5 = tmp.tile([128, 5], F32, tag="m5")
            nc.vector.tensor_scalar(m5[:], ss[:, 0:5], 1.0 / D, RMS_EPS,
                                    ALU.mult, ALU.add)
            ln5 = tmp.tile([128, 5], F32, tag="ln5")
            nc.scalar.activation(ln5[:], m5[:], AF.Ln)
            rs5 = tmp.tile([128, 5], F32, tag="rs5")
            nc.scalar.activation(rs5[:], ln5[:], AF.Exp, scale=-0.5)

            qn = tmp.tile([128, G * D], F32R, tag="qn", bufs=4)
            for h in range(G):
                nc.vector.tensor_scalar_mul(
                    qn[:, h * 64:(h + 1) * 64], qr[:, h * 64:(h + 1) * 64],
                    rs5[:, h:h + 1])
            kn = tmp.tile([128, D], F32R, tag="kn", bufs=4)
            nc.vector.tensor_scalar_mul(kn[:], kr[:], rs5[:, 4:5])

            # transposes run 2 iterations behind so PE never waits on the
            # rope/rms chain of the current block
            qn_kn[tb] = (qn, kn)
            for dtb in ([tb - 2] if tb >= 2 else []) + \
                       ([tb - 1, tb] if tb == TB - 1 else []):
                dqn, dkn = qn_kn.pop(dtb)
                for p in range(2):
                    tq = ps.tile([128, 1024], F32R, tag=("a0", "a1")[p],
                                 name="tq", bufs=1)[:, 0:128]
                    nc.tensor.transpose(tq[:], dqn[:, p * 128:(p + 1) * 128],
                                        identity[:])
                    nc.vector.tensor_copy(
                        qt_sb[p][:, dtb * 128:(dtb + 1) * 128], tq[:])
                tk = ps.tile([128, 1024], F32R, tag="a0",
                             name="tk", bufs=1)[0:64, 0:128]
                nc.tensor.transpose(tk[:], dkn[:], identity[:])
                nc.vector.tensor_copy(kt_sb[0:64, dtb * 128:(dtb + 1) * 128],
                                      tk[:])
                if dtb % 4 == 3:
                    nc.sync.dma_start(
                        kt_sb[64:128, (dtb - 3) * 128:(dtb + 1) * 128],
                        kt_sb[0:64, (dtb - 3) * 128:(dtb + 1) * 128])

        # ========== Phase B + C, interleaved per 512-query chunk ==========
        # Both head-pair streams advance m-by-m in lockstep so the ACT
        # engine (exp) stays saturated; out-projection for the finished
        # chunk is emitted immediately so its PSUM->SBUF copies and output
        # DMAs overlap the next chunk's attention.
        for c in range(NC_):
            ms = list(_active_m(c))
            pvs = [ps.tile([128, 1024], F32, tag=("a0", "a1")[hp],
                           name="pv", bufs=1) for hp in range(2)]
            # order blocks so a full-span m comes first: its PV matmul
            # (start=True) initializes the whole accumulator, letting every
            # later PV run trimmed to its active span without memsets.
            spans = {}
            for m in ms:
                deltas = [4 * c + qpos - m for qpos in range(4)]
                act_q = [q for q in range(4) if 0 <= deltas[q] <= 8]
                spans[m] = (act_q[0], act_q[-1] + 1, deltas)
            mf = next(m for m in ms if spans[m][0] == 0 and spans[m][1] == 4)
            ms_o = [mf] + [m for m in ms if m != mf]
            DEPTH = 2
            pending = {0: [], 1: []}  # hp -> [(p2, mi)] awaiting PV
            for mi in range(len(ms_o) + DEPTH):
                for hp in range(2):
                    if mi < len(ms_o):
                        m = ms_o[mi]
                        qs, qe, deltas = spans[m]
                        sqs, sqe = qs, qe
                        if sqe - sqs == 1:           # N=128 runs at 1/4 rate;
                            if sqs >= 1:             # widen to 256 (even, fast)
                                sqs -= 1
                            else:
                                sqe += 1
                        w = (sqe - sqs) * 128
                        s2 = ps.tile([128, 1024], F32, tag="s", name="s2",
                                     bufs=2)
                        for hl in range(2):
                            o = hl * 512 + sqs * 128
                            nc.tensor.matmul(
                                s2[:, o:o + w],
                                kt_sb[hl * 64:(hl + 1) * 64,
                                      m * 128:(m + 1) * 128],
                                qt_sb[hp][hl * 64:(hl + 1) * 64,
                                          c * 512 + sqs * 128:
                                          c * 512 + sqe * 128],
                                start=True, stop=False,
                                tile_position=(hl * 64, 0),
                                skip_group_check=True)
                            for qpos in range(qs, qe):
                                mt = (mc_sb if deltas[qpos] == 0 else
                                      mw_sb if deltas[qpos] == 8 else None)
                                if mt is None:
                                    continue
                                qo = hl * 512 + qpos * 128
                                nc.tensor.matmul(
                                    s2[:, qo:qo + 128], identity_b[:], mt[:],
                                    start=False, stop=False,
                                    skip_group_check=True)
                        p2 = p2p.tile([128, 1024], F32R)
                        p2v = p2[:].rearrange("p (h f) -> p h f", h=2)
                        s2v = s2[:].rearrange("p (h f) -> p h f", h=2)
                        nc.scalar.activation(
                            p2v[:, :, qs * 128:qe * 128],
                            s2v[:, :, qs * 128:qe * 128],
                            AF.Exp, scale=0.125)
                    if mi >= DEPTH:
                        prev_p2, pmi = pending[hp].pop(0)
                        pm = ms_o[pmi]
                        pqs, pqe, _ = spans[pm]
                        st = (pmi == 0)
                        sp_ = (pmi == len(ms_o) - 1)
                        if st:
                            pqs, pqe = 0, 4
                        pw = (pqe - pqs) * 128
                        for half in range(2):
                            o = half * 512 + pqs * 128
                            nc.tensor.matmul(
                                pvs[hp][:, o:o + pw],
                                (v1a, v1b)[half][:, pm],
                                prev_p2[:, o:o + pw],
                                start=st, stop=sp_, skip_group_check=True)
                    if mi < len(ms_o):
                        pending[hp].append((p2, mi))
            for hp in range(2):
                pv = pvs[hp]
                # denominators: reciprocal straight from PSUM rows 63/64,
                # then partition-broadcast via DMA
                ri = tmp.tile([128, 1024], F32, tag="ri")
                nc.vector.reciprocal_approx_fast(ri[:], pv[:, :])
                rb2 = outs.tile([128, 512], F32, tag="rb2")
                nc.sync.dma_start(
                    rb2[0:64, :],
                    ri[64:65, 0:512].unsqueeze(1).broadcast_to([1, 64, 512]))
                nc.sync.dma_start(
                    rb2[64:128, :],
                    ri[63:64, 512:1024].unsqueeze(1).broadcast_to([1, 64, 512]))
                nc.vector.tensor_tensor(
                    aot[hp][0:64, c * 512:(c + 1) * 512],
                    pv[0:64, 0:512], rb2[0:64, :], ALU.mult)
                nc.vector.tensor_tensor(
                    aot[hp][64:128, c * 512:(c + 1) * 512],
                    pv[64:128, 512:1024], rb2[64:128, :], ALU.mult)
            # out-projection for this chunk; stores batched 4 t-blocks
            # per DMA to cut HWDGE serialization
            for fc in range(2):
                ob4 = outs.tile([128, 4, 512], F32, tag=f"ob{fc}",
                                name=f"ob{fc}", bufs=2)
                for r in range(4):
                    tb = 4 * c + r
                    op = ps.tile([128, 1024], F32, tag=("a1", "a0")[fc],
                                 name="op", bufs=1)[:, 0:512]
                    for k in range(2):
                        nc.tensor.matmul(op[:],
                                         aot[k][:, tb * 128:(tb + 1) * 128],
                                         wo_sb[k][:, fc * 512:(fc + 1) * 512],
                                         start=(k == 0), stop=(k == 1))
                    nc.vector.tensor_copy(ob4[:, r], op[:])
                nc.sync.dma_start(
                    out.rearrange("(cc r p) e -> cc r p e", r=4, p=128)
                       [c, :, :, fc * 512:(fc + 1) * 512]
                       .transpose([1, 0, 2]),
                    ob4[:])

    nc.compile()
    return nc


def _prep_inputs(x, value_embeds, rope_cos, rope_sin, w_qkv, w_gate, w_o):
    cos = np.asarray(rope_cos, np.float32)
    sin = np.asarray(rope_sin, np.float32)
    ropeA = np.concatenate([cos, sin], axis=1)
    ropeB = np.concatenate([-sin, cos], axis=1)
    ii = np.arange(128)[:, None]
    jj = np.arange(128)[None, :]
    import ml_dtypes
    maskC = np.where(ii <= jj, 0.0, -1e30).astype(ml_dtypes.bfloat16)
    maskW = np.where(ii >= jj, 0.0, -1e30).astype(ml_dtypes.bfloat16)
    maps = []
    for core in range(8):
        b, g = divmod(core, 4)
        wq = w_qkv[g * G * D:(g + 1) * G * D]              # [256, E]
        wk = w_qkv[(HQ + g) * D:(HQ + g + 1) * D]          # [64, E]
        wv = w_qkv[(HQ + HK + g) * D:(HQ + HK + g + 1) * D]
        gate_col = np.zeros((2, E), np.float32)
        gate_col[0, :GATE_CH] = w_gate[g]
        wqkvT = np.ascontiguousarray(
            np.concatenate([wq, wk, wv, gate_col], axis=0).T)  # [E, 386]
        maps.append({
            "xT": np.ascontiguousarray(x[b].T),
            "wqkvT": wqkvT,
            "ve3": np.ascontiguousarray(
                3.0 * value_embeds[b, :, g * D:(g + 1) * D]),
            "ropeA": ropeA, "ropeB": ropeB,
            "woT": np.ascontiguousarray(w_o[:, g * G * D:(g + 1) * G * D].T),
            "maskC": maskC, "maskW": maskW,
            "ident": np.eye(128, dtype=np.float32),
            "identb": np.eye(128, dtype=ml_dtypes.bfloat16),
        })
    return maps


def kernel(x, value_embeds, rope_cos, rope_sin, w_qkv, w_gate, w_o,
           trace=False):
    if "nc" not in _CACHE:
        _CACHE["nc"] = build_program()
    nc = _CACHE["nc"]
    in_maps = _prep_inputs(x, value_embeds, rope_cos, rope_sin,
                           w_qkv, w_gate, w_o)
    res = run_bass_kernel_spmd(nc, in_maps, list(range(8)), trace=trace)
    _CACHE["last_exec_time_ns"] = res.exec_time_ns
    out = np.empty((B, T, E), np.float32)
    for b in range(B):
        out[b] = sum(res.results[4 * b + g]["out"] for g in range(4))
    return out



# revision 45
# speedup vs baseline: 1.2665x; 1.2665x over previous
"""Trainium2 Bass kernel for nn_CausalSelfAttention_15178414424258.

GQA sliding-window causal attention (HQ=16, HK=4, D=64, WINDOW=1024) with
value-embedding gating, rope + qk rms-norm, out-projection.

Sharding: tensor-parallel over the 4 kv-head groups x data-parallel over the
2 batches = 8 cores. Each core handles one batch b and one kv group g
(4 q heads, 1 k head, 1 v head), produces a partial out-projection
(its 256 channels of the attention output against the matching w_o columns);
the host sums the 4 partials per batch.

v2.1: bf16 matmuls, token-major PV, one software-pipelined stream.
  A) qkv = x @ w_qkv_shard^T (+ gate logit col) bf16 -> f32 psum, gate via
     exp/recip, v65[tb] = (v + gate*ve) | 1 in bf16, rope (Pool for q, DVE
     for k), rms sums of squares on DVE (square + tensor_reduce), Ln/Exp
     rsqrt on ACT, qn/kn2 bf16, q/k transposed by DMA-transpose (kn written
     twice pre-transpose so kt holds both stationary halves).
  B) per 512-query chunk, two head-pair streams: S^T = k^T q into PSUM f32
     (exact spans), masks added via identity matmuls, exp on ACT -> p2 bf16;
     PV: per (m, head, qblock) matmul with stationary p2 [128,128] and
     moving v65[m] -> token-major [128,64] accumulators + 1-wide denominator
     columns (banks armed once per chunk by a zeros matmul; all PV/den
     matmuls start=False since start=True lazily zeroes the whole bank).
     Normalize = per-partition reciprocal + multiply; aoT via DMA-transpose.
  C) out_partial = aoT^T @ w_oT in 512-wide chunks, bf16 copies to SBUF,
     batched stores. A(c+1) and C(c-1) are interleaved into B(c)'s step
     loop so ACT (exp) and PE stay busy across phase boundaries.
"""
import sys

sys.path.insert(0, "/opt/trn_rl_repo")

from contextlib import ExitStack  # noqa: E402

import numpy as np  # noqa: E402

import concourse.bass as bass  # noqa: E402
import concourse.tile as tile  # noqa: E402
from concourse import bacc, mybir  # noqa: E402
from concourse.bass_utils import run_bass_kernel_spmd  # noqa: E402

F32 = mybir.dt.float32
F32R = mybir.dt.float32r
BF16 = mybir.dt.bfloat16
AF = mybir.ActivationFunctionType
ALU = mybir.AluOpType
AX = mybir.AxisListType

B, T, E = 2, 2048, 1024
HQ, HK, D = 16, 4, 64
WINDOW = 1024
GATE_CH = 12
RMS_EPS = 1e-8
G = HQ // HK          # 4 q heads per kv group
TB = T // 128         # 16 t-blocks
NC_ = 4               # 512-wide query chunks
KT = E // 128         # 8 k-tiles for the qkv matmul

_CACHE = {}

DEBUG = False


def _pin_act_tables(nc):
    """Keep Exp/Ln only in the combined set so insert_act_table_loads
    emits a single table load instead of thrashing between sets."""
    from concourse import hw_specs
    tabs = hw_specs.get_activation_tables(nc.m.arch)
    for name, s in tabs.items():
        if name != "natural_log_exp_and_others":
            s.discard(AF.Exp)
            s.discard(AF.Ln)


def build_program():
    nc = bacc.Bacc("TRN2", target_bir_lowering=False, debug=False, num_devices=8)
    _pin_act_tables(nc)

    xT = nc.declare_dram_parameter("xT", [E, T], BF16, isOutput=False)
    wqkvT = nc.declare_dram_parameter("wqkvT", [E, 386], BF16, isOutput=False)
    ve3 = nc.declare_dram_parameter("ve3", [T, D], F32, isOutput=False)
    ropeA = nc.declare_dram_parameter("ropeA", [T, D], F32, isOutput=False)
    ropeB = nc.declare_dram_parameter("ropeB", [T, D], F32, isOutput=False)
    woT = nc.declare_dram_parameter("woT", [G * D, E], BF16, isOutput=False)
    maskCW = nc.declare_dram_parameter("maskCW", [128, 256], BF16, isOutput=False)
    identb = nc.declare_dram_parameter("identb", [128, 128], BF16, isOutput=False)
    out = nc.declare_dram_parameter("out", [T, E], BF16, isOutput=True)

    with tile.TileContext(nc) as tc, ExitStack() as ctx:
        P = lambda **kw: ctx.enter_context(tc.tile_pool(**kw))
        pers = P(name="pers", bufs=1)
        tmp = P(name="tmp", bufs=2)
        p2p = P(name="p2p", bufs=8)
        outs = P(name="outs", bufs=2)
        # PSUM (8 banks): tag "s" 2x[128,1024] f32 (scores ONLY so the
        # QK->exp rotation never waits on slower consumers, 4 banks),
        # tag "ao" 1x[128,1024] f32 (16 PV accumulators, 2 banks),
        # tag "dn" 1x[128,16] f32 (16 denominator columns, 1 bank),
        # tag "mx" 1x[128,512] f32 (qkv / out-proj / transposes, 1 bank;
        # emissions are spread so each rotation's reader has drained).
        ps = P(name="ps", bufs=1, space="PSUM")

        # ---- persistent SBUF ----
        x_sb = pers.tile([128, KT, T], BF16, tag="x")
        wq_sb = pers.tile([128, KT, 386], BF16, tag="wq")
        wo_sb = pers.tile([128, 2, E], BF16, tag="wo")
        ra_sb = pers.tile([128, TB, D], F32, tag="ra")
        rb_sb = pers.tile([128, TB, D], F32, tag="rb")
        ve_sb = pers.tile([128, TB, D], F32, tag="ve")
        mcw_sb = pers.tile([128, 256], BF16, tag="mcw")
        idb_sb = pers.tile([128, 128], BF16, tag="idb")
        v65 = pers.tile([128, TB, 65], BF16, tag="v65")
        qt_sb = [pers.tile([128, T], BF16, tag=f"qt{p}", name=f"qt{p}")
                 for p in range(2)]
        kt_sb = pers.tile([128, T], BF16, tag="kt")  # k dup'd in both halves
        aot = [pers.tile([128, T], BF16, tag=f"aot{p}", name=f"aot{p}")
               for p in range(2)]

        xT_r = xT.rearrange("(k p) t -> p k t", p=128)
        nc.sync.dma_start(x_sb[:, :, 0:512], xT_r[:, :, 0:512])
        nc.sync.dma_start(wq_sb[:], wqkvT.rearrange("(k p) f -> p k f", p=128))
        for cc in range(1, NC_):
            nc.sync.dma_start(x_sb[:, :, cc * 512:(cc + 1) * 512],
                              xT_r[:, :, cc * 512:(cc + 1) * 512])
        nc.sync.dma_start(wo_sb[:], woT.rearrange("(k p) e -> p k e", p=128))
        nc.sync.dma_start(ra_sb[:], ropeA.rearrange("(tb p) d -> p tb d", p=128))
        nc.sync.dma_start(rb_sb[:], ropeB.rearrange("(tb p) d -> p tb d", p=128))
        nc.sync.dma_start(ve_sb[:], ve3.rearrange("(tb p) d -> p tb d", p=128))
        nc.sync.dma_start(mcw_sb[:], maskCW[:])
        nc.sync.dma_start(idb_sb[:], identb[:])
        nc.vector.memset(v65[:, :, 64:65], 1.0)
        zz = pers.tile([128, 64], BF16, tag="zz")
        nc.vector.memset(zz[:], 0.0)

        if DEBUG:
            dbg_den = nc.declare_dram_parameter("dbg_den", [128, 16], F32,
                                                isOutput=True)

        # ================= emitters =================
        ab_state = {}

        def emit_A_front(tb):
            qkv_ps = ps.tile([128, 1024], F32, tag="s", name="qkv_ps",
                             bufs=2)[:, 0:386]
            for k in range(KT):
                nc.tensor.matmul(qkv_ps[:],
                                 x_sb[:, k, tb * 128:(tb + 1) * 128],
                                 wq_sb[:, k], start=(k == 0),
                                 stop=(k == KT - 1))
            qkv = tmp.tile([128, 386], F32, tag="qkvs", bufs=3)
            nc.vector.tensor_copy(qkv[:], qkv_ps[:])
            # gate logit -> exp on ACT early (only needs the copy)
            eg = tmp.tile([128, 1], F32, tag="eg")
            nc.scalar.activation(eg[:], qkv[:, 384:385], AF.Exp, scale=-1.0)

            # rope into qkr[:, :320] (q on Pool, k on DVE)
            qkr = tmp.tile([128, 320], F32, tag="qkr", bufs=3)

            def rope(dst_off, nh, eng, src_off):
                src = (qkv[:, src_off:src_off + nh * 64]
                       .rearrange("p (h d) -> p h d", h=nh))
                x1 = src[:, :, 0:32].unsqueeze(2).broadcast_to([128, nh, 2, 32])
                x2 = src[:, :, 32:64].unsqueeze(2).broadcast_to([128, nh, 2, 32])
                rav = (ra_sb[:, tb].rearrange("p (two d) -> p two d", two=2)
                       .unsqueeze(1).broadcast_to([128, nh, 2, 32]))
                rbv = (rb_sb[:, tb].rearrange("p (two d) -> p two d", two=2)
                       .unsqueeze(1).broadcast_to([128, nh, 2, 32]))
                dv = (qkr[:, dst_off:dst_off + nh * 64]
                      .rearrange("p (h two d) -> p h two d", h=nh, two=2))
                t1 = tmp.tile([128, nh * 64], F32, tag=f"t1{nh}")
                t1v = t1[:].rearrange("p (h two d) -> p h two d", h=nh, two=2)
                eng.tensor_tensor(t1v, x1, rav, ALU.mult)
                eng.tensor_tensor(dv, x2, rbv, ALU.mult)
                eng.tensor_add(qkr[:, dst_off:dst_off + nh * 64],
                               qkr[:, dst_off:dst_off + nh * 64], t1[:])

            rope(0, G, nc.gpsimd, 0)
            rope(256, 1, nc.vector, 256)

            # rms sums of squares on DVE
            sq = tmp.tile([128, 320], F32, tag="sq")
            nc.vector.tensor_tensor(sq[:], qkr[:], qkr[:], ALU.mult)
            ss = tmp.tile([128, 5], F32, tag="ss")
            nc.vector.tensor_reduce(
                ss[:], sq[:].rearrange("p (h d) -> p h d", h=5), AX.X, ALU.add)
            m5 = tmp.tile([128, 5], F32, tag="m5")
            nc.vector.tensor_scalar(m5[:], ss[:], 1.0 / D, RMS_EPS,
                                    ALU.mult, ALU.add)
            # gate tail (after the DVE rope/sumsq chain to avoid
            # head-of-line stalls waiting on ACT)
            gp = tmp.tile([128, 1], F32, tag="gp")
            nc.vector.tensor_scalar_add(gp[:], eg[:], 1.0)
            gi = tmp.tile([128, 1], F32, tag="gi")
            nc.vector.reciprocal_approx_fast(gi[:], gp[:])
            vt = tmp.tile([128, D], F32, tag="vt")
            nc.gpsimd.tensor_scalar_mul(vt[:], ve_sb[:, tb], gi[:])
            nc.gpsimd.tensor_add(v65[:, tb, 0:64], qkv[:, 320:384], vt[:])
            ab_state[tb] = (qkr, m5)

        def emit_A_back(tb):
            qkr, m5 = ab_state.pop(tb)
            ln5 = tmp.tile([128, 5], F32, tag="ln5")
            nc.scalar.activation(ln5[:], m5[:], AF.Ln)
            rs5 = tmp.tile([128, 5], F32, tag="rs5")
            nc.scalar.activation(rs5[:], ln5[:], AF.Exp, scale=-0.5)
            qn = tmp.tile([128, 256], BF16, tag="qn", bufs=3)
            nc.vector.tensor_tensor(
                qn[:].rearrange("p (h d) -> p h d", h=4),
                qkr[:, 0:256].rearrange("p (h d) -> p h d", h=4),
                rs5[:, 0:4].unsqueeze(2).broadcast_to([128, 4, 64]),
                ALU.mult)
            kn2 = tmp.tile([128, 128], BF16, tag="kn", bufs=3)
            nc.vector.tensor_tensor(
                kn2[:].rearrange("p (two d) -> p two d", two=2),
                qkr[:, 256:320].unsqueeze(1).broadcast_to([128, 2, 64]),
                rs5[:, 4:5].unsqueeze(2).broadcast_to([128, 2, 64]),
                ALU.mult)
            # q/k transposes on the (otherwise idle) SP DMA queue
            for p in range(2):
                nc.sync.dma_start(qt_sb[p][:, tb * 128:(tb + 1) * 128],
                                  qn[:, p * 128:(p + 1) * 128], transpose=True)
            nc.sync.dma_start(kt_sb[:, tb * 128:(tb + 1) * 128], kn2[:],
                              transpose=True)

        ob_tiles = {}

        def emit_C(c, r, fc):
            tb = 4 * c + r
            if r == 0 and fc == 0:
                for f2 in range(2):
                    ob_tiles[(c, f2)] = outs.tile(
                        [128, 4, 512], BF16, tag=f"ob{f2}", name=f"ob{f2}",
                        bufs=2)
            ob = ob_tiles[(c, fc)]
            op = ps.tile([128, 512], F32, tag="op", name="op", bufs=1)
            for k in range(2):
                nc.tensor.matmul(op[:],
                                 aot[k][:, tb * 128:(tb + 1) * 128],
                                 wo_sb[:, k, fc * 512:(fc + 1) * 512],
                                 start=(k == 0), stop=(k == 1))
            nc.vector.tensor_copy(ob[:, r], op[:])
            if r == 3:
                nc.sync.dma_start(
                    out.rearrange("(cc rr p) e -> cc rr p e", rr=4, p=128)
                       [c, :, :, fc * 512:(fc + 1) * 512]
                       .transpose([1, 0, 2]),
                    ob[:])

        def stream(c):
            ms = list(range(max(0, 4 * c - 8), 4 * c + 4))
            ao_ps = ps.tile([128, 1024], F32, tag="ao", name="ao_ps", bufs=1)
            dn_ps = ps.tile([128, 16], F32, tag="dn", name="dn_ps", bufs=1)
            # start=True arms a lazy zero of the whole 2KB bank, so
            # interleaved accumulation groups may not start independently:
            # arm each bank once (writing one full accumulator region so no
            # region is left partially cleared); all PV/den matmuls then
            # accumulate with start=False (first touch after arming fills 0).
            for zdst, zw in ((ao_ps[:, 0:64], 64), (ao_ps[:, 512:576], 64),
                             (dn_ps[:], 16)):
                nc.tensor.matmul(zdst, idb_sb[:], zz[:, 0:zw],
                                 start=True, stop=False, skip_group_check=True)
            spans = {}
            for m in ms:
                deltas = [4 * c + qpos - m for qpos in range(4)]
                act_q = [q for q in range(4) if 0 <= deltas[q] <= 8]
                spans[m] = (act_q[0], act_q[-1] + 1, deltas)
            # interleaved filler: A(c+1) fronts/backs (backs trail fronts by
            # 3 steps so the rms chain has drained) and C(c-1) per fc-half
            extras = {}
            if c < NC_ - 1:
                for j in range(4):
                    extras.setdefault(j, []).append(
                        lambda tb=4 * (c + 1) + j: emit_A_front(tb))
                    extras.setdefault(j + 1, []).append(
                        lambda tb=4 * (c + 1) + j: emit_A_back(tb))
            if c >= 1:
                for r in range(4):
                    extras.setdefault(2 + r, []).append(
                        lambda r=r: (emit_C(c - 1, r, 0), emit_C(c - 1, r, 1)))
            DEPTH = 3
            pending = {0: [], 1: []}  # hp -> [(p2, m)]
            for mi in range(len(ms) + DEPTH):
                for fn in extras.pop(mi, []):
                    fn()
                if mi == len(ms) + DEPTH - 1:
                    for k in sorted(extras):
                        for fn in extras.pop(k):
                            fn()
                for hp in range(2):
                    if mi < len(ms):
                        m = ms[mi]
                        qs, qe, deltas = spans[m]
                        w = (qe - qs) * 128
                        s2 = ps.tile([128, 1024], F32, tag="s", name="s2",
                                     bufs=2)
                        for hl in range(2):
                            o = hl * 512 + qs * 128
                            nc.tensor.matmul(
                                s2[:, o:o + w],
                                kt_sb[hl * 64:(hl + 1) * 64,
                                      m * 128:(m + 1) * 128],
                                qt_sb[hp][hl * 64:(hl + 1) * 64,
                                          c * 512 + qs * 128:
                                          c * 512 + qe * 128],
                                start=True, stop=False,
                                tile_position=(hl * 64, 0),
                                skip_group_check=True)
                            for qpos in range(qs, qe):
                                mo = (0 if deltas[qpos] == 0 else
                                      128 if deltas[qpos] == 8 else None)
                                if mo is None:
                                    continue
                                qo = hl * 512 + qpos * 128
                                nc.tensor.matmul(
                                    s2[:, qo:qo + 128], idb_sb[:],
                                    mcw_sb[:, mo:mo + 128],
                                    start=False, stop=False,
                                    skip_group_check=True)
                        p2 = p2p.tile([128, 1024], BF16)
                        p2v = p2[:].rearrange("p (h f) -> p h f", h=2)
                        s2v = s2[:].rearrange("p (h f) -> p h f", h=2)
                        nc.scalar.activation(
                            p2v[:, :, qs * 128:qe * 128],
                            s2v[:, :, qs * 128:qe * 128],
                            AF.Exp, scale=0.125)
                        pending[hp].append((p2, m))
                    if mi >= DEPTH:
                        pp2, pm = pending[hp].pop(0)
                        pqs, pqe, _ = spans[pm]
                        for hl in range(2):
                            for qpos in range(pqs, pqe):
                                qb = 4 * c + qpos
                                sp_ = (pm == qb)
                                stat = pp2[:, hl * 512 + qpos * 128:
                                           hl * 512 + (qpos + 1) * 128]
                                idx = qpos * 4 + 2 * hp + hl
                                nc.tensor.matmul(
                                    ao_ps[:, idx * 64:(idx + 1) * 64],
                                    stat, v65[:, pm, 0:64],
                                    start=False, stop=sp_,
                                    skip_group_check=True)
                                nc.tensor.matmul(
                                    dn_ps[:, idx:idx + 1],
                                    stat, v65[:, pm, 64:65],
                                    start=False, stop=sp_,
                                    skip_group_check=True)
                if mi >= DEPTH and ms[mi - DEPTH] >= 4 * c:
                    # the diagonal PV for this qblock just drained:
                    # normalize and emit its aoT transposes
                    qpos = ms[mi - DEPTH] - 4 * c
                    tb = 4 * c + qpos
                    if DEBUG and c == 0 and qpos == 3:
                        dd = tmp.tile([128, 16], F32, tag="dd")
                        nc.vector.tensor_copy(dd[:], dn_ps[:])
                        nc.sync.dma_start(dbg_den[:], dd[:])
                    rcp = tmp.tile([128, 4], F32, tag="rcp")
                    nc.vector.reciprocal_approx_fast(
                        rcp[:], dn_ps[:, qpos * 4:qpos * 4 + 4])
                    ao_sb = tmp.tile([128, 256], BF16, tag="ao_sb", bufs=3)
                    nc.vector.tensor_tensor(
                        ao_sb[:].rearrange("p (h d) -> p h d", h=4),
                        ao_ps[:, qpos * 256:(qpos + 1) * 256]
                        .rearrange("p (h d) -> p h d", h=4),
                        rcp[:].unsqueeze(2).broadcast_to([128, 4, 64]),
                        ALU.mult)
                    for p in range(2):
                        nc.sync.dma_start(
                            aot[p][:, tb * 128:(tb + 1) * 128],
                            ao_sb[:, p * 128:(p + 1) * 128], transpose=True)

        # ================= schedule =================
        # Emission order sets scheduler priority (ready instructions run
        # lowest-emission-index first): B(c) is the latency-critical exp
        # chain, so it is emitted before the filler A(c+1)/C(c-1) work of
        # the same super-iteration; the readiness-driven tile scheduler
        # overlaps them.
        emit_A_front(0)
        emit_A_front(1)
        emit_A_back(0)
        emit_A_front(2)
        emit_A_back(1)
        emit_A_front(3)
        emit_A_back(2)
        emit_A_back(3)
        for c in range(NC_):
            stream(c)
        for i in range(8):
            emit_C(NC_ - 1, i // 2, i % 2)

    nc.compile()
    return nc


def _prep_inputs(x, value_embeds, rope_cos, rope_sin, w_qkv, w_gate, w_o):
    import ml_dtypes
    bf = ml_dtypes.bfloat16
    cos = np.asarray(rope_cos, np.float32)
    sin = np.asarray(rope_sin, np.float32)
    ropeA = np.concatenate([cos, sin], axis=1)
    ropeB = np.concatenate([-sin, cos], axis=1)
    ii = np.arange(128)[:, None]
    jj = np.arange(128)[None, :]
    maskC = np.where(ii <= jj, 0.0, -1e30).astype(bf)
    maskW = np.where(ii >= jj, 0.0, -1e30).astype(bf)
    maskCW = np.concatenate([maskC, maskW], axis=1)
    maps = []
    for core in range(8):
        b, g = divmod(core, 4)
        wq = w_qkv[g * G * D:(g + 1) * G * D]              # [256, E]
        wk = w_qkv[(HQ + g) * D:(HQ + g + 1) * D]          # [64, E]
        wv = w_qkv[(HQ + HK + g) * D:(HQ + HK + g + 1) * D]
        gate_col = np.zeros((2, E), np.float32)
        gate_col[0, :GATE_CH] = w_gate[g]
        wqkvT = np.ascontiguousarray(
            np.concatenate([wq, wk, wv, gate_col], axis=0).T)  # [E, 386]
        maps.append({
            "xT": np.ascontiguousarray(x[b].T).astype(bf),
            "wqkvT": wqkvT.astype(bf),
            "ve3": np.ascontiguousarray(
                3.0 * value_embeds[b, :, g * D:(g + 1) * D]),
            "ropeA": ropeA, "ropeB": ropeB,
            "woT": np.ascontiguousarray(
                w_o[:, g * G * D:(g + 1) * G * D].T).astype(bf),
            "maskCW": maskCW,
            "identb": np.eye(128, dtype=bf),
        })
    return maps


def kernel(x, value_embeds, rope_cos, rope_sin, w_qkv, w_gate, w_o,
           trace=False):
    if "nc" not in _CACHE:
        _CACHE["nc"] = build_program()
    nc = _CACHE["nc"]
    in_maps = _prep_inputs(x, value_embeds, rope_cos, rope_sin,
                           w_qkv, w_gate, w_o)
    res = run_bass_kernel_spmd(nc, in_maps, list(range(8)), trace=trace)
    _CACHE["last_exec_time_ns"] = res.exec_time_ns
    out = np.empty((B, T, E), np.float32)
    for b in range(B):
        out[b] = sum(np.asarray(res.results[4 * b + g]["out"],
                                dtype=np.float32) for g in range(4))
    return out
